# revision 6
# baseline (speedup 1.0000x reference)
"""Multi-head attention Trainium2 kernel (Bass/Tile, SPMD over 8 cores).

fp16 compute, fp32 PSUM accumulation. Rel err vs fp32 reference ~1e-3.
Sharding: data parallel over batch. Core i computes batches [2i, 2i+2).

Structure:
  - Softmax normalization on HOST: kernel ships numerator and denominator
    (ones-column rides along the PV matmul) as fp16; host divides +
    transposes + concats heads.
  - PV matmul: lhsT = P^T chunk (K=t 128, M=s 128), rhs = v_aug (N=66).
  - Score/exp stagger: the Activation engine needs 1.23us per exp tile
    (4 tiles/item = 93% busy), so bunched score matmuls always stall on
    the 2-slot score PSUM ring. Fix: spread the 4 ST pairs of each item
    across a whole iteration - t2/t3 of the previous item are embedded
    inside the q/k projection matmul streams (they accumulate in other
    PSUM banks, so interleaving is legal), t0/t1 sit between the PV
    halves. The first 10 ST tiles ride inside vproj(1), where the
    Activation engine is otherwise idle.
  - DEPTH=3 lookahead: pv(k) consumes exps finished >=1.5 iterations
    earlier, including for the last items (no exposed exp tail).
  - PSUM: stp = 2 x 2-bank tiles (row-packed score pairs, one wide exp
    per t). psq = 4 x 1-bank ring for q/k projections + PV outputs.
    vproj(0) rounds take 2 groups from an stp tile + 2 psq slots;
    vproj(1) uses 4 psq slots (stp is carrying embedded STs by then).
  - PE warm-up: 32 scratch matmuls bridge the start barrier -> first
    data arrival so real work starts at speed.
  - Input DMA, two queues, first-use order: sync: wv c0 half0 (gates
    first matmul), xt[b0], wk, xt[b1]; gpsimd: wv half0 c1-7, wq,
    wv half1. Out-DMAs alternate sync/gpsimd except the last two items
    (sync only, so the end-of-NEFF gpsimd drain has nothing in flight).
"""

import numpy as np

import concourse.bass as bass
import concourse.mybir as mybir
import concourse.tile as tile
from concourse.bass_utils import run_bass_kernel_spmd

B, S, D, H, DH = 16, 512, 1024, 16, 64
N_CORES = 8
B_LOC = B // N_CORES  # 2 batches per core
C = D // 128  # 8 contraction chunks over d
TC = S // 128  # 4 chunks over s/t
HP = H // 2  # 8 head pairs
EA = DH + 2  # 64 e cols + ones col + pad
F32 = mybir.dt.float32
FP16 = mybir.dt.float16
SCALE = 1.0 / np.sqrt(np.float32(D))
EXP_BIAS = -5.0  # exp(logit-5): keeps P in fp16 range; cancels in normalize
DEPTH = 3
N_ITEMS = B_LOC * HP  # 16


def legalize_waits(nc, cap=1):
    """This walrus build supports at most `cap` sync-wait commands per
    instruction; hoist excess waits onto preceding same-engine NoOps."""
    n_split = 0
    for f in nc.m.functions:
        for blk in f.blocks:
            new_insts = []
            for inst in blk.instructions:
                si = getattr(inst, "sync_info", None)
                waits = list(si.on_wait) if si is not None and si.on_wait else []
                if len(waits) > cap:
                    keep, rest = waits[:cap], waits[cap:]
                    while rest:
                        chunk, rest = rest[:cap], rest[cap:]
                        nop = mybir.InstNoOp(
                            name=f"I-waitsplit-{nc.next_id()}", ins=[], outs=[]
                        )
                        nop.engine = inst.engine
                        nop.sync_info = mybir.SyncInfo(on_wait=chunk, on_update=[])
                        nc.register_instruction(nop, overwrite=True)
                        new_insts.append(nop)
                        n_split += 1
                    si.on_wait = keep
                new_insts.append(inst)
            blk.instructions[:] = new_insts
    return n_split


def build_program():
    nc = bass.Bass()
    xt_d = nc.declare_dram_parameter("xt", [B_LOC, C, 128, S], FP16, isOutput=False)
    wq_d = nc.declare_dram_parameter("wq", [C, 128, D], FP16, isOutput=False)
    wk_d = nc.declare_dram_parameter("wk", [C, 128, D], FP16, isOutput=False)
    wv_d = nc.declare_dram_parameter("wv", [C, 128, D], FP16, isOutput=False)
    # numerator^T + denominator, partition-major: [b, pair, s%128, half, s//128, e]
    out_d = nc.declare_dram_parameter(
        "out", [B_LOC, HP, 128, 2, TC, EA], FP16, isOutput=True
    )

    with tile.TileContext(nc) as tc:
        with (
            tc.tile_pool(name="wpool", bufs=1) as wpool,
            tc.tile_pool(name="xpool", bufs=1) as xpool,
            tc.tile_pool(name="vpool", bufs=8) as vpool,
            tc.tile_pool(name="qkpool", bufs=5) as qkpool,
            tc.tile_pool(name="ppool", bufs=18) as ppool,
            tc.tile_pool(name="opool", bufs=8) as opool,
            tc.tile_pool(name="cpool", bufs=1) as cpool,
            tc.tile_pool(name="psq", bufs=4, space="PSUM") as psq,
            tc.tile_pool(name="stp", bufs=2, space="PSUM") as stp,
        ):
            # scratch for PE warm-up; memset on gpsimd (its queue is free
            # earliest) so warm-up matmuls start right after the barrier
            scratch = cpool.tile([128, 128], FP16, tag="scratch", bufs=1)
            nc.gpsimd.memset(scratch, 0.001)
            exp_bias = cpool.tile([128, 1], F32, tag="expbias", bufs=1)
            nc.vector.memset(exp_bias, EXP_BIAS)

            # ---- input DMAs in first-use order on two queues ----
            xts = [
                [
                    xpool.tile([128, S], FP16, tag=f"xt{b}_{c}", name=f"xt{b}_{c}")
                    for c in range(C)
                ]
                for b in range(B_LOC)
            ]
            wq_sb = [
                wpool.tile([128, D], FP16, tag=f"wq{c}", name=f"wq{c}")
                for c in range(C)
            ]
            wk_sb = [
                wpool.tile([128, D], FP16, tag=f"wk{c}", name=f"wk{c}")
                for c in range(C)
            ]
            wv_sb = [
                wpool.tile([128, D], FP16, tag=f"wv{c}", name=f"wv{c}")
                for c in range(C)
            ]
            # wv c0 half0 gates the very first vproj matmul: lead sync with it
            nc.sync.dma_start(out=wv_sb[0][:, 0:512], in_=wv_d[0][:, 0:512])
            for c in range(C):
                nc.sync.dma_start(out=xts[0][c], in_=xt_d[0, c])
            for c in range(1, C):
                nc.gpsimd.dma_start(out=wv_sb[c][:, 0:512], in_=wv_d[c][:, 0:512])
            for c in range(C):
                nc.sync.dma_start(out=wk_sb[c], in_=wk_d[c])
                nc.gpsimd.dma_start(out=wq_sb[c], in_=wq_d[c])
            for c in range(C):
                nc.sync.dma_start(out=xts[1][c], in_=xt_d[1, c])
                nc.gpsimd.dma_start(out=wv_sb[c][:, 512:1024], in_=wv_d[c][:, 512:1024])

            # ---- PE clock warm-up on scratch data while first DMAs fly ----
            for w in range(32):
                wps = psq.tile([128, 512], F32, tag="ps", name=f"warm{w}")
                nc.tensor.matmul(
                    wps[:, 0:128], lhsT=scratch, rhs=scratch, start=True, stop=True
                )

            # V_aug layout [128(t), h, 64(e) + ones + pad]
            vaugs = {}
            for b in range(B_LOC):
                vaugs[b] = [
                    vpool.tile([128, H, EA], FP16, tag=f"vaug{b}", name=f"vaug{b}_{t}")
                    for t in range(TC)
                ]
                for t in range(TC):
                    nc.vector.memset(vaugs[b][t][:, :, DH : DH + 2], 1.0)

            items = [(b, p) for b in range(B_LOC) for p in range(HP)]
            qk = {}  # item -> (qt, kt)
            pts = {j: [None] * TC for j in range(N_ITEMS)}  # item -> 4 exp tiles

            def emit_st(j, t):
                """Row-packed score pair + one wide exp for (item j, chunk t)."""
                qt, kt = qk[j]
                ps2 = stp.tile([128, 2, 512], F32, tag="st", name=f"st{j}_{t}")
                for half in range(2):
                    lo, hi = 64 * half, 64 * (half + 1)
                    nc.tensor.matmul(
                        ps2[:, half, :],
                        lhsT=kt[lo:hi, t * 128 : (t + 1) * 128],
                        rhs=qt[lo:hi, :],
                        start=True,
                        stop=True,
                    )
                pt = ppool.tile([128, 2, 512], FP16, tag="p", name=f"p{j}_{t}")
                nc.scalar.activation(
                    pt.rearrange("p a b -> p (a b)"),
                    ps2.rearrange("p a b -> p (a b)"),
                    mybir.ActivationFunctionType.Exp,
                    scale=float(SCALE),
                    bias=exp_bias[:, :],
                )
                pts[j][t] = pt

            def emit_proj(j, w_sb, tag, embed=None):
                """Projection of item j through w; optionally embeds the ST
                pair `embed` after chunk 3 (different PSUM banks, legal)."""
                b, p = items[j]
                dst = qkpool.tile([128, S], FP16, tag=tag, name=f"{tag}{j}")
                ps = psq.tile([128, 512], F32, tag="ps")
                for c in range(C):
                    nc.tensor.matmul(
                        ps,
                        lhsT=w_sb[c][:, p * 128 : (p + 1) * 128],
                        rhs=xts[b][c],
                        start=(c == 0),
                        stop=(c == C - 1),
                    )
                    if c == 3 and embed is not None:
                        emit_st(*embed)
                nc.vector.tensor_copy(dst, ps)
                return dst

            def vproj0():
                # chunk-major: 4 t-groups live per half-round; two groups in
                # one stp tile (separate banks) + two psq slots, so each
                # pool's reuse distance is a full round.
                b = 0
                for half in range(2):
                    st2 = stp.tile([128, 2, 512], F32, tag="st", name=f"vst{half}")
                    groups = [st2[:, 0, :], st2[:, 1, :]] + [
                        psq.tile([128, 512], F32, tag="ps", name=f"vp{half}{t}")
                        for t in range(2)
                    ]
                    for c in range(C):
                        for t in range(TC):
                            nc.tensor.matmul(
                                groups[t],
                                lhsT=xts[b][c][:, t * 128 : (t + 1) * 128],
                                rhs=wv_sb[c][:, half * 512 : (half + 1) * 512],
                                start=(c == 0),
                                stop=(c == C - 1),
                            )
                    for t in range(TC):
                        nc.vector.tensor_copy(
                            vaugs[b][t][:, half * 8 : (half + 1) * 8, 0:DH],
                            groups[t].rearrange("p (h e) -> p h e", h=8),
                        )

            def vproj1(embeds):
                # psq-only groups (stp carries the embedded score pairs);
                # one embedded ST every 6 matmuls spreads exps at ~1.3us,
                # matching the Activation engine's 1.23us/tile rate.
                b = 1
                queue = list(embeds)
                mmcount = 0
                for half in range(2):
                    groups = [
                        psq.tile([128, 512], F32, tag="ps", name=f"v1p{half}{t}")
                        for t in range(TC)
                    ]
                    for c in range(C):
                        for t in range(TC):
                            nc.tensor.matmul(
                                groups[t],
                                lhsT=xts[b][c][:, t * 128 : (t + 1) * 128],
                                rhs=wv_sb[c][:, half * 512 : (half + 1) * 512],
                                start=(c == 0),
                                stop=(c == C - 1),
                            )
                            mmcount += 1
                            if mmcount % 6 == 0 and queue:
                                emit_st(*queue.pop(0))
                    for t in range(TC):
                        nc.vector.tensor_copy(
                            vaugs[b][t][:, half * 8 : (half + 1) * 8, 0:DH],
                            groups[t].rearrange("p (h e) -> p h e", h=8),
                        )
                assert not queue

            def pv_half(k, half):
                b, p = items[k]
                h = p * 2 + half
                pso_t = psq.tile([128, TC, EA], F32, tag="ps", name=f"pv{k}{half}")
                # sc outer / t inner: each psum accumulation group must
                # be a contiguous matmul sequence within its bank (the
                # 4 sc-slices share one bank, so t-outer interleaving
                # corrupts accumulation — verified on HW)
                for sc in range(TC):
                    for t in range(TC):
                        nc.tensor.matmul(
                            pso_t[:, sc, :],
                            lhsT=pts[k][t][:, half, sc * 128 : (sc + 1) * 128],
                            rhs=vaugs[b][t][:, h, :],
                            start=(t == 0),
                            stop=(t == TC - 1),
                        )
                osb = opool.tile([128, TC, EA], FP16, tag="osb", name=f"o{k}_{half}")
                nc.vector.tensor_copy(
                    osb.rearrange("p a b -> p (a b)"),
                    pso_t.rearrange("p a b -> p (a b)"),
                )
                # last two items: keep both halves on sync so the end-of-NEFF
                # gpsimd drain has nothing in flight
                eng = nc.gpsimd if (half == 1 and k < N_ITEMS - 2) else nc.sync
                eng.dma_start(out=out_d[b, p, :, half], in_=osb)

            # ---- schedule ----
            vproj0()
            for j in range(DEPTH):
                qk[j] = (
                    emit_proj(j, wq_sb, "qt"),
                    emit_proj(j, wk_sb, "kt"),
                )
            vproj1(
                [(0, 0), (0, 1), (0, 2), (0, 3), (1, 0), (1, 1), (1, 2), (1, 3)]
                + [(2, 0), (2, 1)]
            )
            for k in range(N_ITEMS):
                j = k + DEPTH
                if j < N_ITEMS:
                    qt = emit_proj(j, wq_sb, "qt", embed=(j - 1, 2))
                    kt = emit_proj(j, wk_sb, "kt", embed=(j - 1, 3))
                    qk[j] = (qt, kt)
                    emit_st(j, 0)
                    pv_half(k, 0)
                    emit_st(j, 1)
                    pv_half(k, 1)
                elif j == N_ITEMS:
                    emit_st(N_ITEMS - 1, 2)
                    emit_st(N_ITEMS - 1, 3)
                    pv_half(k, 0)
                    pv_half(k, 1)
                else:
                    pv_half(k, 0)
                    pv_half(k, 1)

    legalize_waits(nc)
    return nc


def _prep_inputs(x, Wq, Wk, Wv):
    x = np.ascontiguousarray(np.asarray(x, dtype=np.float32))
    # x [B, S, D] -> per-core xT [B_LOC, C, 128, S]
    xt = x.reshape(N_CORES, B_LOC, S, D).transpose(0, 1, 3, 2)
    xt = np.ascontiguousarray(xt).reshape(N_CORES, B_LOC, C, 128, S).astype(np.float16)
    wp = []
    for W in (Wq, Wk, Wv):
        W = np.asarray(W, dtype=np.float32)
        # [H, D, DH] -> [D, H*DH] (d-major) -> [C, 128, H*DH]
        wp.append(
            np.ascontiguousarray(W.transpose(1, 0, 2))
            .reshape(C, 128, H * DH)
            .astype(np.float16)
        )
    return xt, wp[0], wp[1], wp[2]


_PROGRAM = None


def _get_program():
    global _PROGRAM
    if _PROGRAM is None:
        _PROGRAM = build_program()
    return _PROGRAM


def _finalize(raw):
    """raw: [B_LOC, HP, 128, 2, TC, EA] fp16 per core -> [B_LOC, S, D] fp32."""
    raw = raw.astype(np.float32)
    num = raw[..., :DH]  # [b, p, j, half, sc, e]
    den = raw[..., DH]  # [b, p, j, half, sc]
    o = num / den[..., None]
    # [b, p, j, half, sc, e] -> [b, sc, j, p, half, e] -> [b, s, d]
    return np.ascontiguousarray(o.transpose(0, 4, 2, 1, 3, 5)).reshape(B_LOC, S, D)


def run(x, Wq, Wk, Wv, trace=False, nc=None):
    xt, wq_p, wk_p, wv_p = _prep_inputs(x, Wq, Wk, Wv)
    if nc is None:
        nc = _get_program()
    in_maps = [
        {"xt": xt[i], "wq": wq_p, "wk": wk_p, "wv": wv_p} for i in range(N_CORES)
    ]
    res = run_bass_kernel_spmd(nc, in_maps, list(range(N_CORES)), trace=trace)
    out = np.concatenate(
        [_finalize(res.results[i]["out"]) for i in range(N_CORES)], axis=0
    )
    return out, res


def kernel(x, Wq, Wk, Wv):
    out, _ = run(x, Wq, Wk, Wv, trace=False)
    return out


# revision 7
# speedup vs baseline: 1.0560x; 1.0560x over previous
"""Multi-head attention Trainium2 kernel (Bass/Tile, SPMD over 8 cores).

fp16 compute, fp32 PSUM accumulation. Rel err vs fp32 reference ~1e-3.
Sharding: data parallel over batch. Core i computes batches [2i, 2i+2).

Structure:
  - Softmax normalization on HOST: kernel ships numerator and denominator
    (ones-column rides along the PV matmul) as fp16; host divides +
    transposes + concats heads.
  - PV matmul: lhsT = P^T chunk (K=t 128, M=s 128), rhs = v_aug (N=66).
  - Score/exp stagger: the Activation engine needs 1.23us per exp tile
    (4 tiles/item = 93% busy), so bunched score matmuls always stall on
    the 2-slot score PSUM ring. Fix: spread the 4 ST pairs of each item
    across a whole iteration - t2/t3 of the previous item are embedded
    inside the q/k projection matmul streams (they accumulate in other
    PSUM banks, so interleaving is legal), t0/t1 sit between the PV
    halves. The first 10 ST tiles ride inside vproj(1), where the
    Activation engine is otherwise idle.
  - DEPTH=3 lookahead: pv(k) consumes exps finished >=1.5 iterations
    earlier, including for the last items (no exposed exp tail).
  - PSUM: stp = 2 x 2-bank tiles (row-packed score pairs, one wide exp
    per t). psq = 4 x 1-bank ring for q/k projections + PV outputs.
    vproj(0) rounds take 2 groups from an stp tile + 2 psq slots;
    vproj(1) uses 4 psq slots (stp is carrying embedded STs by then).
  - PE warm-up: 32 scratch matmuls bridge the start barrier -> first
    data arrival so real work starts at speed.
  - Input DMA, two queues, first-use order: sync: wv c0 half0 (gates
    first matmul), xt[b0], wk, xt[b1]; gpsimd: wv half0 c1-7, wq,
    wv half1. Out-DMAs alternate sync/gpsimd except the last two items
    (sync only, so the end-of-NEFF gpsimd drain has nothing in flight).
"""

import numpy as np

import concourse.bass as bass
import concourse.mybir as mybir
import concourse.tile as tile
from concourse.bass_utils import run_bass_kernel_spmd

B, S, D, H, DH = 16, 512, 1024, 16, 64
N_CORES = 8
B_LOC = B // N_CORES  # 2 batches per core
C = D // 128  # 8 contraction chunks over d
TC = S // 128  # 4 chunks over s/t
HP = H // 2  # 8 head pairs
EA = DH + 2  # 64 e cols + ones col + pad
F32 = mybir.dt.float32
FP16 = mybir.dt.float16
SCALE = 1.0 / np.sqrt(np.float32(D))
EXP_BIAS = -5.0  # exp(logit-5): keeps P in fp16 range; cancels in normalize
DEPTH = 3
N_ITEMS = B_LOC * HP  # 16


def legalize_waits(nc, cap=1):
    """This walrus build supports at most `cap` sync-wait commands per
    instruction; hoist excess waits onto preceding same-engine NoOps."""
    n_split = 0
    for f in nc.m.functions:
        for blk in f.blocks:
            new_insts = []
            for inst in blk.instructions:
                si = getattr(inst, "sync_info", None)
                waits = list(si.on_wait) if si is not None and si.on_wait else []
                if len(waits) > cap:
                    keep, rest = waits[:cap], waits[cap:]
                    while rest:
                        chunk, rest = rest[:cap], rest[cap:]
                        nop = mybir.InstNoOp(
                            name=f"I-waitsplit-{nc.next_id()}", ins=[], outs=[]
                        )
                        nop.engine = inst.engine
                        nop.sync_info = mybir.SyncInfo(on_wait=chunk, on_update=[])
                        nc.register_instruction(nop, overwrite=True)
                        new_insts.append(nop)
                        n_split += 1
                    si.on_wait = keep
                new_insts.append(inst)
            blk.instructions[:] = new_insts
    return n_split


def build_program():
    nc = bass.Bass()
    xt_d = nc.declare_dram_parameter("xt", [B_LOC, C, 128, S], FP16, isOutput=False)
    wq_d = nc.declare_dram_parameter("wq", [C, 128, D], FP16, isOutput=False)
    wk_d = nc.declare_dram_parameter("wk", [C, 128, D], FP16, isOutput=False)
    wv_d = nc.declare_dram_parameter("wv", [C, 128, D], FP16, isOutput=False)
    # numerator^T + denominator, partition-major: [b, pair, s%128, half, s//128, e]
    out_d = nc.declare_dram_parameter(
        "out", [B_LOC, HP, 128, 2, TC, EA], FP16, isOutput=True
    )

    with tile.TileContext(nc) as tc:
        with (
            tc.tile_pool(name="wpool", bufs=1) as wpool,
            tc.tile_pool(name="xpool", bufs=1) as xpool,
            tc.tile_pool(name="vpool", bufs=8) as vpool,
            tc.tile_pool(name="qkpool", bufs=5) as qkpool,
            tc.tile_pool(name="ppool", bufs=16) as ppool,
            tc.tile_pool(name="opool", bufs=8) as opool,
            tc.tile_pool(name="cpool", bufs=1) as cpool,
            tc.tile_pool(name="psq", bufs=4, space="PSUM") as psq,
            tc.tile_pool(name="stp", bufs=2, space="PSUM") as stp,
        ):
            # scratch for PE warm-up; memset on gpsimd (its queue is free
            # earliest) so warm-up matmuls start right after the barrier
            scratch = cpool.tile([128, 128], FP16, tag="scratch", bufs=1)
            nc.gpsimd.memset(scratch, 0.001)
            exp_bias = cpool.tile([128, 1], F32, tag="expbias", bufs=1)
            nc.vector.memset(exp_bias, EXP_BIAS)

            # ---- input DMAs in first-use order on two queues ----
            xts = [
                [
                    xpool.tile([128, S], FP16, tag=f"xt{b}_{c}", name=f"xt{b}_{c}")
                    for c in range(C)
                ]
                for b in range(B_LOC)
            ]
            wq_sb = [
                wpool.tile([128, D], FP16, tag=f"wq{c}", name=f"wq{c}")
                for c in range(C)
            ]
            wk_sb = [
                wpool.tile([128, D], FP16, tag=f"wk{c}", name=f"wk{c}")
                for c in range(C)
            ]
            wv_sb = [
                wpool.tile([128, D], FP16, tag=f"wv{c}", name=f"wv{c}")
                for c in range(C)
            ]
            # wv c0 half0 gates the very first vproj matmul: lead sync with it
            nc.sync.dma_start(out=wv_sb[0][:, 0:512], in_=wv_d[0][:, 0:512])
            for c in range(C):
                nc.sync.dma_start(out=xts[0][c], in_=xt_d[0, c])
            for c in range(1, C):
                nc.gpsimd.dma_start(out=wv_sb[c][:, 0:512], in_=wv_d[c][:, 0:512])
            for c in range(C):
                nc.sync.dma_start(out=xts[1][c], in_=xt_d[1, c])
                nc.gpsimd.dma_start(out=wv_sb[c][:, 512:1024], in_=wv_d[c][:, 512:1024])
            for c in range(C):
                nc.gpsimd.dma_start(out=wq_sb[c], in_=wq_d[c])
            for c in range(C):
                nc.gpsimd.dma_start(out=wk_sb[c], in_=wk_d[c])

            # ---- PE clock warm-up on scratch data while first DMAs fly ----
            for w in range(32):
                wps = psq.tile([128, 512], F32, tag="ps", name=f"warm{w}")
                nc.tensor.matmul(
                    wps[:, 0:128], lhsT=scratch, rhs=scratch, start=True, stop=True
                )

            # V_aug layout [128(t), h, 64(e) + ones + pad]
            vaugs = {}
            for b in range(B_LOC):
                vaugs[b] = [
                    vpool.tile([128, H, EA], FP16, tag=f"vaug{b}", name=f"vaug{b}_{t}")
                    for t in range(TC)
                ]
                for t in range(TC):
                    nc.vector.memset(vaugs[b][t][:, :, DH : DH + 2], 1.0)

            items = [(b, p) for b in range(B_LOC) for p in range(HP)]
            qk = {}  # item -> (qt, kt)
            pts = {j: [None] * TC for j in range(N_ITEMS)}  # item -> 4 exp tiles

            def emit_st(j, t):
                """Row-packed score pair + one wide exp for (item j, chunk t)."""
                qt, kt = qk[j]
                ps2 = stp.tile([128, 2, 512], F32, tag="st", name=f"st{j}_{t}")
                for half in range(2):
                    lo, hi = 64 * half, 64 * (half + 1)
                    nc.tensor.matmul(
                        ps2[:, half, :],
                        lhsT=kt[lo:hi, t * 128 : (t + 1) * 128],
                        rhs=qt[lo:hi, :],
                        start=True,
                        stop=True,
                    )
                pt = ppool.tile([128, 2, 512], FP16, tag="p", name=f"p{j}_{t}")
                nc.scalar.activation(
                    pt.rearrange("p a b -> p (a b)"),
                    ps2.rearrange("p a b -> p (a b)"),
                    mybir.ActivationFunctionType.Exp,
                    scale=float(SCALE),
                    bias=exp_bias[:, :],
                )
                pts[j][t] = pt

            def emit_proj(j, w_sb, tag, embed=None, embed_end=None):
                """Projection of item j through w; optionally embeds ST
                pairs after chunk 3 and after the chunk loop (they
                accumulate in other PSUM banks, so interleaving is legal)."""
                b, p = items[j]
                dst = qkpool.tile([128, S], FP16, tag=tag, name=f"{tag}{j}")
                ps = psq.tile([128, 512], F32, tag="ps")
                for c in range(C):
                    nc.tensor.matmul(
                        ps,
                        lhsT=w_sb[c][:, p * 128 : (p + 1) * 128],
                        rhs=xts[b][c],
                        start=(c == 0),
                        stop=(c == C - 1),
                    )
                    if c == 3 and embed is not None:
                        emit_st(*embed)
                if embed_end is not None:
                    emit_st(*embed_end)
                nc.vector.tensor_copy(dst, ps)
                return dst

            def vproj0():
                # chunk-major: 4 t-groups live per half-round; two groups in
                # one stp tile (separate banks) + two psq slots, so each
                # pool's reuse distance is a full round.
                b = 0
                for half in range(2):
                    st2 = stp.tile([128, 2, 512], F32, tag="st", name=f"vst{half}")
                    groups = [st2[:, 0, :], st2[:, 1, :]] + [
                        psq.tile([128, 512], F32, tag="ps", name=f"vp{half}{t}")
                        for t in range(2)
                    ]
                    for c in range(C):
                        for t in range(TC):
                            nc.tensor.matmul(
                                groups[t],
                                lhsT=xts[b][c][:, t * 128 : (t + 1) * 128],
                                rhs=wv_sb[c][:, half * 512 : (half + 1) * 512],
                                start=(c == 0),
                                stop=(c == C - 1),
                            )
                    for t in range(TC):
                        nc.vector.tensor_copy(
                            vaugs[b][t][:, half * 8 : (half + 1) * 8, 0:DH],
                            groups[t].rearrange("p (h e) -> p h e", h=8),
                        )

            def vproj1():
                b = 1
                for half in range(2):
                    st2 = stp.tile([128, 2, 512], F32, tag="st", name=f"v1st{half}")
                    groups = [st2[:, 0, :], st2[:, 1, :]] + [
                        psq.tile([128, 512], F32, tag="ps", name=f"v1p{half}{t}")
                        for t in range(2)
                    ]
                    for c in range(C):
                        for t in range(TC):
                            nc.tensor.matmul(
                                groups[t],
                                lhsT=xts[b][c][:, t * 128 : (t + 1) * 128],
                                rhs=wv_sb[c][:, half * 512 : (half + 1) * 512],
                                start=(c == 0),
                                stop=(c == C - 1),
                            )
                    for t in range(TC):
                        nc.vector.tensor_copy(
                            vaugs[b][t][:, half * 8 : (half + 1) * 8, 0:DH],
                            groups[t].rearrange("p (h e) -> p h e", h=8),
                        )

            def pv_half(k, half):
                b, p = items[k]
                h = p * 2 + half
                pso_t = psq.tile([128, TC, EA], F32, tag="ps", name=f"pv{k}{half}")
                # sc outer / t inner: each psum accumulation group must
                # be a contiguous matmul sequence within its bank (the
                # 4 sc-slices share one bank, so t-outer interleaving
                # corrupts accumulation — verified on HW)
                for sc in range(TC):
                    for t in range(TC):
                        nc.tensor.matmul(
                            pso_t[:, sc, :],
                            lhsT=pts[k][t][:, half, sc * 128 : (sc + 1) * 128],
                            rhs=vaugs[b][t][:, h, :],
                            start=(t == 0),
                            stop=(t == TC - 1),
                        )
                osb = opool.tile([128, TC, EA], FP16, tag="osb", name=f"o{k}_{half}")
                nc.vector.tensor_copy(
                    osb.rearrange("p a b -> p (a b)"),
                    pso_t.rearrange("p a b -> p (a b)"),
                )
                # last two items: keep both halves on sync so the end-of-NEFF
                # gpsimd drain has nothing in flight
                eng = nc.gpsimd if (half == 1 and k < N_ITEMS - 2) else nc.sync
                eng.dma_start(out=out_d[b, p, :, half], in_=osb)

            # ---- schedule ----
            # iteration n: projections of item n with the four score pairs
            # of item n-1 embedded (their qt/kt casts finished last
            # iteration, so the casts are off the score critical chain),
            # plus the PV halves of item n-3.
            vproj0()
            vproj1()
            for n in range(N_ITEMS + DEPTH):
                if n < N_ITEMS:
                    e = [(n - 1, t) for t in range(TC)] if n >= 1 else [None] * TC
                    qt = emit_proj(n, wq_sb, "qt", embed=e[0], embed_end=e[1])
                    if n >= DEPTH:
                        pv_half(n - DEPTH, 0)
                    kt = emit_proj(n, wk_sb, "kt", embed=e[2], embed_end=e[3])
                    qk[n] = (qt, kt)
                    if n >= DEPTH:
                        pv_half(n - DEPTH, 1)
                elif n == N_ITEMS:
                    emit_st(N_ITEMS - 1, 0)
                    emit_st(N_ITEMS - 1, 1)
                    pv_half(n - DEPTH, 0)
                    emit_st(N_ITEMS - 1, 2)
                    emit_st(N_ITEMS - 1, 3)
                    pv_half(n - DEPTH, 1)
                else:
                    pv_half(n - DEPTH, 0)
                    pv_half(n - DEPTH, 1)

    legalize_waits(nc)
    return nc


def _prep_inputs(x, Wq, Wk, Wv):
    x = np.ascontiguousarray(np.asarray(x, dtype=np.float32))
    # x [B, S, D] -> per-core xT [B_LOC, C, 128, S]
    xt = x.reshape(N_CORES, B_LOC, S, D).transpose(0, 1, 3, 2)
    xt = np.ascontiguousarray(xt).reshape(N_CORES, B_LOC, C, 128, S).astype(np.float16)
    wp = []
    for W in (Wq, Wk, Wv):
        W = np.asarray(W, dtype=np.float32)
        # [H, D, DH] -> [D, H*DH] (d-major) -> [C, 128, H*DH]
        wp.append(
            np.ascontiguousarray(W.transpose(1, 0, 2))
            .reshape(C, 128, H * DH)
            .astype(np.float16)
        )
    return xt, wp[0], wp[1], wp[2]


_PROGRAM = None


def _get_program():
    global _PROGRAM
    if _PROGRAM is None:
        _PROGRAM = build_program()
    return _PROGRAM


def _finalize(raw):
    """raw: [B_LOC, HP, 128, 2, TC, EA] fp16 per core -> [B_LOC, S, D] fp32."""
    raw = raw.astype(np.float32)
    num = raw[..., :DH]  # [b, p, j, half, sc, e]
    den = raw[..., DH]  # [b, p, j, half, sc]
    o = num / den[..., None]
    # [b, p, j, half, sc, e] -> [b, sc, j, p, half, e] -> [b, s, d]
    return np.ascontiguousarray(o.transpose(0, 4, 2, 1, 3, 5)).reshape(B_LOC, S, D)


def run(x, Wq, Wk, Wv, trace=False, nc=None):
    xt, wq_p, wk_p, wv_p = _prep_inputs(x, Wq, Wk, Wv)
    if nc is None:
        nc = _get_program()
    in_maps = [
        {"xt": xt[i], "wq": wq_p, "wk": wk_p, "wv": wv_p} for i in range(N_CORES)
    ]
    res = run_bass_kernel_spmd(nc, in_maps, list(range(N_CORES)), trace=trace)
    out = np.concatenate(
        [_finalize(res.results[i]["out"]) for i in range(N_CORES)], axis=0
    )
    return out, res


def kernel(x, Wq, Wk, Wv):
    out, _ = run(x, Wq, Wk, Wv, trace=False)
    return out
